# revision 1
# baseline (speedup 1.0000x reference)
"""Trainium2 Bass kernel for nn_Affinity (gnn_message_passing).

M[(a,b),(c,d)] = sum_{j,i} H2[a,j]H2[c,j] H1[b,i]H1[d,i] Me[j,i] + diag(Mp).

M is [5184, 5184] f32, block-sparse: block (a,c) is nonzero only when a==c or
(a,c) is an edge of graph 2. Strategy: shard output rows (a-bands) across the
8 cores; on each core compute only the nonzero [72,72] blocks via matmuls
(fully dense math is ~6x more PE work), zero-fill the core's output slab with
big HWDGE DMAs, and scatter the computed blocks with one indirect (SWDGE) DMA
per bundle using host-precomputed flat indices. All index-derived tables
(incidence matrices, selection matrices, scatter indices) are host-built and
passed as per-core inputs; every floating-point op runs on device.
"""
import sys
sys.path.insert(0, '/opt/trn_rl_repo')
import math
import numpy as np

N = 72
E = 288
D = 64
NC = 8
W = N * N          # 5184
GROUP_ROWS = 3 * N  # 216 rows per output tensor
PAD_IDX = 2 ** 30

F32 = None
I32 = None


def _split_waits(nc, limit=1):
    """This walrus build rejects instructions with >limit sem waits; move the
    excess onto same-engine NoOps inserted immediately before (same bb order =
    same engine program order, so semantics are preserved)."""
    import concourse.mybir as mybir
    for f in nc.m.functions:
        for bb in f.blocks:
            new_insts = []
            for inst in bb.instructions:
                si = inst.sync_info
                waits = list(si.on_wait) if si and si.on_wait else []
                if len(waits) > limit:
                    extra, keep = waits[:-limit], waits[-limit:]
                    for i in range(0, len(extra), limit):
                        nop = mybir.InstNoOp(
                            name=nc.get_next_instruction_name(),
                            engine=inst.engine, ins=[], outs=[],
                            sync_info=mybir.SyncInfo(
                                on_wait=extra[i:i + limit], on_update=[]),
                        )
                        nc.register_instruction(nop)
                        new_insts.append(nop)
                    si.on_wait = keep
                new_insts.append(inst)
            bb.instructions[:] = new_insts


def _incidence(src, dst):
    H = np.zeros((N, E), np.float32)
    H[src, np.arange(E)] = 1.0
    H[dst, np.arange(E)] = 1.0
    return H


def _plan_assignment(src2, dst2):
    """Balance a-bands across cores and 3 groups/core; K = bundle capacity."""
    nbrs = [set() for _ in range(N)]
    for s, d in zip(src2, dst2):
        nbrs[int(s)].add(int(d))
        nbrs[int(d)].add(int(s))
    deg = [len(x) for x in nbrs]
    order = sorted(range(N), key=lambda a: -deg[a])
    cores = [[] for _ in range(NC)]
    loads = [0] * NC
    for a in order:
        c = min((c for c in range(NC) if len(cores[c]) < 9), key=lambda c: loads[c])
        cores[c].append(a)
        loads[c] += deg[a]
    plans = []
    K = 2
    for c in range(NC):
        bands_sorted = sorted(cores[c], key=lambda a: -deg[a])
        groups = [[] for _ in range(3)]
        gl = [0] * 3
        for a in bands_sorted:
            g = min((g for g in range(3) if len(groups[g]) < 3), key=lambda g: gl[g])
            groups[g].append(a)
            gl[g] += deg[a]
        for g in range(3):
            K = max(K, 1 + math.ceil(gl[g] / 3))
        plans.append(groups)
    return plans, nbrs, K


def _build_tables(plans, nbrs, K, H2):
    """Per-core (SELT [E,9K] f32, IDX [9K,1] i32, OH [N,9] f32, band_list).

    IDX[slot] = flat element offset of the block's row in the BLOCK-TILED
    group tensor [216, 5184]: row (la_in_group*72 + c), i.e. offset
    (la_in_group*72 + c) * 5184. Padding slots get PAD_IDX (skipped via
    bounds_check)."""
    tables = []
    for c in range(NC):
        groups = plans[c]
        band_list = groups[0] + groups[1] + groups[2]
        SELT = np.zeros((E, 9 * K), np.float32)
        IDX = np.full((9 * K, 1), PAD_IDX, np.int32)
        OH = np.zeros((N, 9), np.float32)
        for l in range(9):
            a = band_list[l]
            OH[a, l] = 1.0
            lb = l % 3
            SELT[:, l * K] = H2[a] * H2[a]
            IDX[l * K, 0] = (lb * N + a) * W
        for g in range(3):
            blocks = []
            for j, a in enumerate(groups[g]):
                for cc in sorted(nbrs[a]):
                    blocks.append((a, cc, j))
            slots = [(3 * g + i, k) for k in range(1, K) for i in range(3)]
            assert len(blocks) <= len(slots)
            for (a, cc, j), (l, k) in zip(blocks, slots):
                SELT[:, l * K + k] = H2[a] * H2[cc]
                IDX[l * K + k, 0] = (j * N + cc) * W
        tables.append((SELT, IDX, OH, band_list))
    return tables


def _pack_fields(K):
    return [("U1T", D, 72), ("U2T", D, 72), ("OH", 72, 9)]


def _packb_fields(K):
    KC = 9 * K

    def pad(w):
        return (w + 63) // 64 * 64
    return [("H1T0", 96, pad(N)), ("H1T1", 96, pad(N)), ("H1T2", 96, pad(N)),
            ("F1", 72, pad(D)), ("H1", 72, pad(E)), ("F2", 72, pad(D)),
            ("L1T", D, pad(D)), ("L2T", D, pad(D)), ("S2", 72, pad(E)),
            ("D2M", 72, pad(E)),
            ("SELT0", 96, pad(KC)), ("SELT1", 96, pad(KC)), ("SELT2", 96, pad(KC))]


def _pack_offsets(K):
    offs = {}
    pw = 0
    for nm, r, w in _pack_fields(K):
        offs[nm] = pw
        pw += w
    return offs


def _pack_width(K):
    return sum(w for _, _, w in _pack_fields(K))


def _packb_offsets(K):
    offs = {}
    pw = 0
    for nm, r, w in _packb_fields(K):
        offs[nm] = pw
        pw += w
    return offs


def _packb_width(K):
    return sum(w for _, _, w in _packb_fields(K))


def _build_nc(K, zero_fill=False):
    import concourse.bass as bass
    import concourse.mybir as mybir
    import concourse.tile as tile
    from concourse.masks import make_identity

    F32 = mybir.dt.float32
    BF16 = mybir.dt.bfloat16
    I32 = mybir.dt.int32
    KC = 9 * K

    nc = bass.Bass()
    pack_d = nc.declare_dram_parameter("PACK", [96, _pack_width(K)], F32,
                                       isOutput=False)
    packb_d = nc.declare_dram_parameter("PACKB", [96, _packb_width(K)], BF16, isOutput=False)
    idx_d = nc.declare_dram_parameter("IDX3", [3 * K, 3], I32, isOutput=False)
    out_d = [nc.declare_dram_parameter(f"out{g}", [GROUP_ROWS, W], F32, isOutput=True)
             for g in range(3)]
    # internal DRAM bounce for the b<->k axis swap, stored k-major (the
    # permuted, small-segment write happens early and hidden; the read back
    # into block-per-partition SBUF layout is fully contiguous)
    scratch_d = [nc.dram_tensor(f"restage_scratch{g}", [3 * K, N * N], F32)
                 for g in range(3)]

    offs = _pack_offsets(K)
    offsb = _packb_offsets(K)

    with tile.TileContext(nc) as tc:
        with tc.tile_pool(name="cst", bufs=1) as cst, \
             tc.tile_pool(name="wrk", bufs=2) as wrk, \
             tc.tile_pool(name="stg", bufs=3) as stg, \
             tc.tile_pool(name="ps", bufs=2, space="PSUM") as ps, \
             tc.tile_pool(name="psb", bufs=5, space="PSUM") as psb:

            if zero_fill:
                zt = cst.tile([128, N * W // 128], F32)
                nc.vector.memset(zt[:], 0.0)
                for g in range(3):
                    for j in range(3):
                        dst = out_d[g][j * N:(j + 1) * N, :] \
                            .rearrange("b w -> (b w)") \
                            .rearrange("(p f) -> p f", p=128)
                        eng = nc.sync if (g * 3 + j) % 2 == 0 else nc.scalar
                        eng.dma_start(out=dst, in_=zt[:])

            # ---- input loads: 3 DMAs total ----
            pk = cst.tile([96, _pack_width(K)], F32)
            nc.sync.dma_start(out=pk[:], in_=pack_d[:])
            pkb = cst.tile([96, _packb_width(K)], BF16)
            nc.scalar.dma_start(out=pkb[:], in_=packb_d[:])
            idx3 = cst.tile([3 * K, 3], I32)
            nc.sync.dma_start(out=idx3[:], in_=idx_d[:])

            def fld(nm, r, w):
                return pk[0:r, offs[nm]:offs[nm] + w]

            def fldb(nm, r, w):
                return pkb[0:r, offsb[nm]:offsb[nm] + w]
            u1t = fld("U1T", D, 72)
            u2t = fld("U2T", D, 72)
            oh = fld("OH", 72, 9)
            f1 = fldb("F1", 72, D)
            h1 = fldb("H1", 72, E)
            f2 = fldb("F2", 72, D)
            l1t = fldb("L1T", D, D)
            l2t = fldb("L2T", D, D)
            s2 = fldb("S2", 72, E)
            d2m = fldb("D2M", 72, E)
            selt = [fldb(f"SELT{jc}", 96, KC) for jc in range(3)]
            h1tb = [fldb(f"H1T{ic}", 96, N) for ic in range(3)]

            ident = cst.tile([N, N], F32)
            make_identity(nc, ident[:])
            identb = cst.tile([N, N], BF16)
            nc.gpsimd.tensor_copy(out=identb[:], in_=ident[:])

            # ---- Mp / diag tiles ----
            mp_p = ps.tile([N, N], F32, tag="mmp")
            nc.tensor.matmul(out=mp_p[:], lhsT=u1t, rhs=u2t, start=True, stop=True)
            mp = wrk.tile([N, N], F32, tag="mp")
            nc.vector.tensor_copy(out=mp[:], in_=mp_p[:])
            mpsel_p = ps.tile([N, 9], F32, tag="mmp")
            nc.tensor.matmul(out=mpsel_p[:], lhsT=mp[:], rhs=oh, start=True, stop=True)
            mpsel = wrk.tile([N, 9], F32, tag="mpsel")
            nc.vector.tensor_copy(out=mpsel[:], in_=mpsel_p[:])
            dls = []
            for l in range(9):
                dlb = wrk.tile([N, N], BF16, name=f"dlb{l}", tag=f"dlb{l}")
                nc.scalar.activation(
                    out=dlb[:], in_=ident[:],
                    func=mybir.ActivationFunctionType.Copy,
                    scale=mpsel[:, l:l + 1])
                dls.append(dlb)

            # ---- Me / w chain ----
            r1t = wrk.tile([D, D], BF16, tag="r1t")
            nc.vector.tensor_relu(out=r1t[:], in_=l1t)
            r2t = wrk.tile([D, D], BF16, tag="r2t")
            nc.vector.tensor_relu(out=r2t[:], in_=l2t)

            fs_p = ps.tile([D, E], F32, tag="mmp")
            nc.tensor.matmul(out=fs_p[:], lhsT=f2, rhs=s2, start=True, stop=True)
            fs = wrk.tile([D, E], BF16, tag="fs")
            nc.vector.tensor_copy(out=fs[:], in_=fs_p[:])
            fd_p = ps.tile([D, E], F32, tag="mmp")
            nc.tensor.matmul(out=fd_p[:], lhsT=f2, rhs=d2m, start=True, stop=True)
            fdt = wrk.tile([D, E], BF16, tag="fdt")
            nc.vector.tensor_copy(out=fdt[:], in_=fd_p[:])

            z1t_p = ps.tile([D, E], F32, tag="mmp")
            nc.tensor.matmul(out=z1t_p[:], lhsT=f1, rhs=h1, start=True, stop=True)
            z1t = wrk.tile([D, E], BF16, tag="z1t")
            nc.vector.tensor_copy(out=z1t[:], in_=z1t_p[:])

            vv_p = ps.tile([D, E], F32, tag="mmp")
            nc.tensor.matmul(out=vv_p[:], lhsT=r1t[:], rhs=fs[:], start=True, stop=False)
            nc.tensor.matmul(out=vv_p[:], lhsT=r2t[:], rhs=fdt[:], start=False, stop=True)
            vv = wrk.tile([D, E], BF16, tag="vv")
            nc.vector.tensor_copy(out=vv[:], in_=vv_p[:])

            me = []
            for jc in range(3):
                me_p = ps.tile([96, E], F32, tag="mmp")
                nc.tensor.matmul(out=me_p[:], lhsT=z1t[:, 96 * jc:96 * (jc + 1)],
                                 rhs=vv[:], start=True, stop=True)
                me_c = wrk.tile([96, E], BF16, name=f"me{jc}", tag=f"me{jc}")
                nc.vector.tensor_copy(out=me_c[:], in_=me_p[:])
                me.append(me_c)

            wt = []
            for ic in range(3):
                wt_p = ps.tile([96, KC], F32, tag="mmp")
                for jc in range(3):
                    nc.tensor.matmul(out=wt_p[:], lhsT=me[jc][:, 96 * ic:96 * (ic + 1)],
                                     rhs=selt[jc], start=(jc == 0), stop=(jc == 2))
                wt_c = wrk.tile([96, KC], BF16, name=f"wtb{ic}", tag=f"wtb{ic}")
                nc.scalar.copy(out=wt_c[:], in_=wt_p[:])
                wt.append(wt_c)

            # ---- Stage A: 9 merged rhs builds (bundle-triple x ic),
            # split DVE (6) / GpSimd (3) ----
            rc3 = []
            for t in range(3):
                row = []
                for ic in range(3):
                    rc = cst.tile([96, 3 * K * N], BF16, name=f"rc{t}_{ic}",
                                  tag=f"rc{t}_{ic}")
                    eng = nc.vector  # gpsimd SBUF access locks out DVE
                    eng.tensor_tensor(
                        out=rc[:].rearrange("p (k d) -> p k d", d=N),
                        in0=wt[ic][:, 3 * K * t:3 * K * (t + 1)][:, :, None]
                            .to_broadcast([96, 3 * K, N]),
                        in1=h1tb[ic][:, None, :].to_broadcast([96, 3 * K, N]),
                        op=mybir.AluOpType.mult)
                    row.append(rc)
                rc3.append(row)

            # ---- Stage B: block matmuls + staged copies (ACT) ----
            stageds = []
            ngroups = [(s, min(s + 7, K)) for s in range(0, K, 7)]
            for l in range(9):
                staged = stg.tile([N, K * N], F32, name=f"staged{l}", tag=f"staged{l}")
                for (ks, ke) in ngroups:
                    bp = psb.tile([N, (ke - ks) * N], F32, tag="bp")
                    for ic in range(3):
                        last = (ic == 2) and ks != 0
                        nc.tensor.matmul(
                            out=bp[:], lhsT=h1tb[ic],
                            rhs=rc3[l // 3][ic][:, ((l % 3) * K + ks) * N:
                                                ((l % 3) * K + ke) * N],
                            start=(ic == 0), stop=last)
                    if ks == 0:
                        nc.tensor.matmul(out=bp[:, 0:N], lhsT=identb[:],
                                         rhs=dls[l][:], start=False, stop=True)
                    nc.scalar.copy(out=staged[:, ks * N:ke * N], in_=bp[:])
                stageds.append(staged)

            # ---- Stage C/D: per group, write the three bundles' staged
            # tiles to the group's scratch (k-major: the permuted small-segment
            # writes spread across all three DMA paths), then a contiguous
            # read-back and the group's indirect scatter, pipelined per group.
            restaged = cst.tile([9 * K, N * N], F32)
            rings = (nc.sync, nc.scalar, nc.gpsimd)
            for g in range(3):
                for j in range(3):
                    l = 3 * g + j
                    rings[j].dma_start(
                        out=scratch_d[g][j * K:(j + 1) * K, :].rearrange(
                            "k (b d) -> b k d", d=N),
                        in_=stageds[l][:].rearrange("b (k d) -> b k d", d=N))
            for g in range(3):
                rings[g % 2].dma_start(
                    out=restaged[3 * K * g:3 * K * (g + 1), :],
                    in_=scratch_d[g][:])
                nc.gpsimd.indirect_dma_start(
                    out=out_d[g][:],
                    out_offset=bass.IndirectOffsetOnAxis(ap=idx3[:, g:g + 1], axis=1),
                    in_=restaged[3 * K * g:3 * K * (g + 1), :],
                    in_offset=None,
                    bounds_check=GROUP_ROWS * W - 1,
                    oob_is_err=False)

    _split_waits(nc)
    return nc


def _prepare(inputs):
    import ml_dtypes
    ins = {k: np.asarray(v) for k, v in inputs.items()}
    F1 = ins["F1"].astype(np.float32)
    F2 = ins["F2"].astype(np.float32)
    U1 = ins["U1"].astype(np.float32)
    U2 = ins["U2"].astype(np.float32)
    l1 = ins["lamda1"].astype(np.float32)
    l2 = ins["lamda2"].astype(np.float32)
    src1 = ins["src1"].astype(np.int64)
    dst1 = ins["dst1"].astype(np.int64)
    src2 = ins["src2"].astype(np.int64)
    dst2 = ins["dst2"].astype(np.int64)

    H1 = _incidence(src1, dst1)
    H2 = _incidence(src2, dst2)
    S2 = np.zeros((N, E), np.float32)
    S2[src2, np.arange(E)] = 1.0
    D2M = np.zeros((N, E), np.float32)
    D2M[dst2, np.arange(E)] = 1.0

    plans, nbrs, K = _plan_assignment(src2, dst2)
    tables = _build_tables(plans, nbrs, K, H2)

    offs = _pack_offsets(K)
    offsb = _packb_offsets(K)
    base = np.zeros((96, _pack_width(K)), np.float32)

    def put(nm, arr):
        r, w = arr.shape
        base[0:r, offs[nm]:offs[nm] + w] = arr
    put("U1T", np.ascontiguousarray(U1.T))
    put("U2T", np.ascontiguousarray(U2.T))
    PACKB = np.zeros((96, _packb_width(K)), ml_dtypes.bfloat16)

    def putb(nm, arr):
        r, w = arr.shape
        PACKB[0:r, offsb[nm]:offsb[nm] + w] = arr.astype(ml_dtypes.bfloat16)
    for ic in range(3):
        putb(f"H1T{ic}", H1.T[96 * ic:96 * (ic + 1), :])
    putb("F1", F1)
    putb("H1", H1)
    putb("F2", F2)
    putb("L1T", np.ascontiguousarray(l1.T))
    putb("L2T", np.ascontiguousarray(l2.T))
    putb("S2", S2)
    putb("D2M", D2M)

    in_maps = []
    band_lists = []
    for c in range(NC):
        SELT, IDX, OH, band_list = tables[c]
        pack = base.copy()
        pack[0:72, offs["OH"]:offs["OH"] + 9] = OH
        packb = PACKB.copy()
        for jc in range(3):
            arr = SELT[96 * jc:96 * (jc + 1), :]
            packb[0:96, offsb[f"SELT{jc}"]:offsb[f"SELT{jc}"] + arr.shape[1]] = \
                arr.astype(ml_dtypes.bfloat16)
        IDX3 = np.ascontiguousarray(IDX.reshape(3, 3 * K).T).astype(np.int32)
        in_maps.append({"PACK": pack, "PACKB": packb, "IDX3": IDX3})
        band_lists.append(band_list)
    return in_maps, band_lists, K


_CACHE = {}


def kernel(**inputs):
    from concourse.bass_utils import run_bass_kernel_spmd

    in_maps, band_lists, K = _prepare(inputs)
    nc = _CACHE.get(K)
    if nc is None:
        nc = _build_nc(K)
        _CACHE[K] = nc
    res = run_bass_kernel_spmd(nc, in_maps, list(range(NC)))
    M = np.empty((N * N, N * N), np.float32)
    for c in range(NC):
        outs = res.results[c]
        for l in range(9):
            a = band_lists[c][l]
            g, j = l // 3, l % 3
            # out_g is block-tiled: [3(j), 72(c), 72(b), 72(d)]
            band = outs[f"out{g}"].reshape(3, N, N, N)[j]          # [c, b, d]
            M[a * N:(a + 1) * N, :] = band.transpose(1, 0, 2).reshape(N, N * N)
    return M



# revision 3
# speedup vs baseline: 2.4893x; 2.4893x over previous
"""Trainium2 Bass kernel for nn_Affinity (gnn_message_passing).

M[(a,b),(c,d)] = sum_{j,i} H2[a,j]H2[c,j] H1[b,i]H1[d,i] W[j,i] + diag(Mp).

Structure exploited:
 - Nonzero blocks (a,c) of M: a==c or (a,c) an edge of graph 2 -> "slots".
   626 slots total, balanced 9 bands/core across 8 cores (<=79 slots/core).
 - Within a block, support is graph-1 adjacency + diagonal: only 622 of 5184
   (b,d) positions can be nonzero, the SAME set for every slot.
 - Per-slot weights w_s[i] = sum_q Me[q,i] SELT[q,s] factor through
   ZS = Xsum^T SELT so the edge-affinity matrix Me is never materialized.
 - Block values OUT[s, f] = sum_i wt[i,s] KRS[i, f] where
   KRS[i,(b,d)] = H1[b,i]H1[d,i] is a host-built 0/1 Khatri-Rao table
   restricted to the support -> one 3-pass matmul chain with partition=slot,
   so results DMA straight out with no transpose/scatter.
 - diag(Mp) folds in as a 4th PSUM accumulation against a [I72|0] table.

All index-derived tables (incidence, SELT, KRS, OHSS, IDPAD) are host-built;
every floating-point op runs on device. Host assembly only places computed
values (and zeros) into the full [5184, 5184] output.
"""
import sys
sys.path.insert(0, '/opt/trn_rl_repo')
import numpy as np

N = 72
E = 288
D = 64
NC = 8
SPAD = 80          # padded slots per core (max observed 79)
NSUP_PAD = 640     # padded support columns (622 actual), 2 chunks of 320
CH = 320           # PSUM free-chunk width


def _split_waits(nc, limit=1):
    """This walrus build rejects instructions with >limit sem waits; move the
    excess onto same-engine NoOps inserted immediately before (same bb order =
    same engine program order, so semantics are preserved)."""
    import concourse.mybir as mybir
    for f in nc.m.functions:
        for bb in f.blocks:
            new_insts = []
            for inst in bb.instructions:
                si = inst.sync_info
                waits = list(si.on_wait) if si and si.on_wait else []
                if len(waits) > limit:
                    extra, keep = waits[:-limit], waits[-limit:]
                    for i in range(0, len(extra), limit):
                        nop = mybir.InstNoOp(
                            name=nc.get_next_instruction_name(),
                            engine=inst.engine, ins=[], outs=[],
                            sync_info=mybir.SyncInfo(
                                on_wait=extra[i:i + limit], on_update=[]),
                        )
                        nc.register_instruction(nop)
                        new_insts.append(nop)
                    si.on_wait = keep
                new_insts.append(inst)
            bb.instructions[:] = new_insts


def _incidence(src, dst):
    H = np.zeros((N, E), np.float32)
    H[src, np.arange(E)] = 1.0
    H[dst, np.arange(E)] = 1.0
    return H


def _neighbors(src, dst):
    nbrs = [set() for _ in range(N)]
    for s, d in zip(src, dst):
        nbrs[int(s)].add(int(d))
        nbrs[int(d)].add(int(s))
    return nbrs


def _plan_assignment(nbrs2):
    """9 bands per core, greedily balancing slot count (1 + deg per band)."""
    deg = [len(x) for x in nbrs2]
    order = sorted(range(N), key=lambda a: -deg[a])
    cores = [[] for _ in range(NC)]
    loads = [0] * NC
    for a in order:
        c = min((c for c in range(NC) if len(cores[c]) < 9),
                key=lambda c: loads[c])
        cores[c].append(a)
        loads[c] += 1 + deg[a]
    assert max(loads) <= SPAD
    return cores


_FIELDS = [("f1", 72, D), ("f2", 72, D), ("h1", 72, E), ("s2", 72, E),
           ("d2m", 72, E), ("l1t", D, D), ("l2t", D, D), ("u1t", D, 72),
           ("u2t", D, 72), ("ohss", 72, SPAD), ("idpad", 72, CH),
           ("selt0", 96, SPAD), ("selt1", 96, SPAD), ("selt2", 96, SPAD)]


def _pk_offs():
    offs, pw = {}, 0
    for nm, r, w in _FIELDS:
        offs[nm] = pw
        pw += w
    return offs, pw


def _build_nc():
    import concourse.bass as bass
    import concourse.mybir as mybir
    import concourse.tile as tile

    F32 = mybir.dt.float32
    BF16 = mybir.dt.bfloat16

    offs, PW = _pk_offs()

    nc = bass.Bass()
    packb_d = nc.declare_dram_parameter("PACKB", [96, PW], BF16, isOutput=False)
    krs_d = nc.declare_dram_parameter("KRSD", [96, 3 * NSUP_PAD], BF16,
                                      isOutput=False)
    out_d = nc.declare_dram_parameter("OUT", [SPAD, NSUP_PAD], F32,
                                      isOutput=True)

    with tile.TileContext(nc) as tc:
        with tc.tile_pool(name="cst", bufs=1) as cst, \
             tc.tile_pool(name="ps", bufs=3, space="PSUM") as ps, \
             tc.tile_pool(name="psb", bufs=2, space="PSUM") as psb:

            pkb = cst.tile([96, PW], BF16)
            nc.sync.dma_start(out=pkb[:], in_=packb_d[:])
            krs = cst.tile([96, 3 * NSUP_PAD], BF16)
            nc.scalar.dma_start(out=krs[:], in_=krs_d[:])

            def fld(nm):
                r, w = next((r, w) for n, r, w in _FIELDS if n == nm)
                return pkb[0:r, offs[nm]:offs[nm] + w]

            f1, f2, h1 = fld("f1"), fld("f2"), fld("h1")
            s2, d2m = fld("s2"), fld("d2m")
            l1t, l2t = fld("l1t"), fld("l2t")
            u1t, u2t = fld("u1t"), fld("u2t")
            ohss, idpad = fld("ohss"), fld("idpad")
            selt = [fld(f"selt{q}") for q in range(3)]

            # relu(lamda^T) tiles
            r1t = cst.tile([D, D], BF16, tag="r1t")
            nc.vector.tensor_relu(out=r1t[:], in_=l1t)
            r2t = cst.tile([D, D], BF16, tag="r2t")
            nc.vector.tensor_relu(out=r2t[:], in_=l2t)

            # Xsum chunks: z1tT[q][p, d] = sum_n H1[n, 96q+p] F1[n, d]
            z1tc = []
            for q in range(3):
                zp = ps.tile([96, D], F32, tag="mm")
                nc.tensor.matmul(out=zp[:], lhsT=h1[:, 96 * q:96 * (q + 1)],
                                 rhs=f1, start=True, stop=True)
                zc = cst.tile([96, D], BF16, tag=f"z1tc{q}")
                if q % 2 == 0:
                    nc.vector.tensor_copy(out=zc[:], in_=zp[:])
                else:
                    nc.scalar.copy(out=zc[:], in_=zp[:])
                z1tc.append(zc)

            # fs = F2^T S2, fdt = F2^T D2M  [D, E]
            fs_p = ps.tile([D, E], F32, tag="mm")
            nc.tensor.matmul(out=fs_p[:], lhsT=f2, rhs=s2, start=True, stop=True)
            fsc = cst.tile([D, E], BF16, tag="fsc")
            nc.scalar.copy(out=fsc[:], in_=fs_p[:])
            fd_p = ps.tile([D, E], F32, tag="mm")
            nc.tensor.matmul(out=fd_p[:], lhsT=f2, rhs=d2m, start=True, stop=True)
            fdc = cst.tile([D, E], BF16, tag="fdc")
            nc.vector.tensor_copy(out=fdc[:], in_=fd_p[:])

            # Mp = U1 U2^T  [72, 72]
            mp_p = ps.tile([72, 72], F32, tag="mm")
            nc.tensor.matmul(out=mp_p[:], lhsT=u1t, rhs=u2t, start=True, stop=True)
            mpc = cst.tile([72, 72], BF16, tag="mpc")
            nc.scalar.copy(out=mpc[:], in_=mp_p[:])

            # vv = relu(L1) fs + relu(L2) fdt  [D, E]
            vv_p = ps.tile([D, E], F32, tag="mm")
            nc.tensor.matmul(out=vv_p[:], lhsT=r1t[:], rhs=fsc[:],
                             start=True, stop=False)
            nc.tensor.matmul(out=vv_p[:], lhsT=r2t[:], rhs=fdc[:],
                             start=False, stop=True)
            vvc = cst.tile([D, E], BF16, tag="vvc")
            nc.vector.tensor_copy(out=vvc[:], in_=vv_p[:])

            # ZS[d, s] = sum_q Xsum[q, d] SELT[q, s]  [D, SPAD]
            zs_p = ps.tile([D, SPAD], F32, tag="mm")
            for q in range(3):
                nc.tensor.matmul(out=zs_p[:], lhsT=z1tc[q][:], rhs=selt[q],
                                 start=(q == 0), stop=(q == 2))
            zsc = cst.tile([D, SPAD], BF16, tag="zsc")
            nc.scalar.copy(out=zsc[:], in_=zs_p[:])

            # mpx[p, s] = Mp[a_s, p] for diag slots (cols of OHSS), else 0
            mpx_p = ps.tile([72, SPAD], F32, tag="mm")
            nc.tensor.matmul(out=mpx_p[:], lhsT=mpc[:], rhs=ohss,
                             start=True, stop=True)
            mpxc = cst.tile([72, SPAD], BF16, tag="mpxc")
            nc.scalar.copy(out=mpxc[:], in_=mpx_p[:])

            # wt[ic][x, s] = sum_d vv[d, 96ic+x] ZS[d, s]  [96, SPAD]
            wtc = []
            for ic in range(3):
                wt_p = ps.tile([96, SPAD], F32, tag="mm")
                nc.tensor.matmul(out=wt_p[:],
                                 lhsT=vvc[:, 96 * ic:96 * (ic + 1)],
                                 rhs=zsc[:], start=True, stop=True)
                wc = cst.tile([96, SPAD], BF16, tag=f"wtc{ic}")
                if ic % 2 == 0:
                    nc.vector.tensor_copy(out=wc[:], in_=wt_p[:])
                else:
                    nc.scalar.copy(out=wc[:], in_=wt_p[:])
                wtc.append(wc)

            # block values: OUT[s, f] = sum_i wt[i, s] KRS[i, f] (+ Mp diag)
            staged = cst.tile([SPAD, NSUP_PAD], F32)
            for ch in range(2):
                bp = psb.tile([SPAD, CH], F32, tag="bp")
                for ic in range(3):
                    nc.tensor.matmul(
                        out=bp[:], lhsT=wtc[ic][:],
                        rhs=krs[0:96, ic * NSUP_PAD + ch * CH:
                                ic * NSUP_PAD + (ch + 1) * CH],
                        start=(ic == 0), stop=(ic == 2 and ch != 0))
                if ch == 0:
                    # diag(Mp) add: rhs = [I72 | 0], lhsT = mpx
                    nc.tensor.matmul(out=bp[:], lhsT=mpxc[:], rhs=idpad,
                                     start=False, stop=True)
                if ch == 0:
                    nc.vector.tensor_copy(
                        out=staged[:, ch * CH:(ch + 1) * CH], in_=bp[:])
                else:
                    nc.scalar.copy(
                        out=staged[:, ch * CH:(ch + 1) * CH], in_=bp[:])

            nc.sync.dma_start(out=out_d[:], in_=staged[:])

    _split_waits(nc)
    return nc


def _prepare(inputs):
    import ml_dtypes
    ins = {k: np.asarray(v) for k, v in inputs.items()}
    F1 = ins["F1"].astype(np.float32)
    F2 = ins["F2"].astype(np.float32)
    U1 = ins["U1"].astype(np.float32)
    U2 = ins["U2"].astype(np.float32)
    l1 = ins["lamda1"].astype(np.float32)
    l2 = ins["lamda2"].astype(np.float32)
    src1 = ins["src1"].astype(np.int64)
    dst1 = ins["dst1"].astype(np.int64)
    src2 = ins["src2"].astype(np.int64)
    dst2 = ins["dst2"].astype(np.int64)

    H1 = _incidence(src1, dst1)
    H2 = _incidence(src2, dst2)
    S2 = np.zeros((N, E), np.float32)
    S2[src2, np.arange(E)] = 1.0
    D2M = np.zeros((N, E), np.float32)
    D2M[dst2, np.arange(E)] = 1.0

    nbrs2 = _neighbors(src2, dst2)
    nbrs1 = _neighbors(src1, dst1)
    cores = _plan_assignment(nbrs2)

    # support: diag first (col b = (b,b)), then off-diag adjacency pairs
    supp = [(b, b) for b in range(N)]
    for b in range(N):
        for d in sorted(nbrs1[b]):
            supp.append((b, d))
    nsup = len(supp)
    assert nsup <= NSUP_PAD
    KRS = np.zeros((E, NSUP_PAD), np.float32)
    for f, (b, d) in enumerate(supp):
        KRS[:, f] = H1[b] * H1[d]
    KRSD = np.zeros((96, 3 * NSUP_PAD), ml_dtypes.bfloat16)
    for ic in range(3):
        KRSD[:, ic * NSUP_PAD:(ic + 1) * NSUP_PAD] = \
            KRS[96 * ic:96 * (ic + 1)].astype(ml_dtypes.bfloat16)

    offs, PW = _pk_offs()
    base = np.zeros((96, PW), ml_dtypes.bfloat16)

    def put(arr, nm, r=None):
        rr, w = arr.shape
        base[0:rr, offs[nm]:offs[nm] + w] = arr.astype(ml_dtypes.bfloat16)
    put(F1, "f1")
    put(F2, "f2")
    put(H1, "h1")
    put(S2, "s2")
    put(D2M, "d2m")
    put(np.ascontiguousarray(l1.T), "l1t")
    put(np.ascontiguousarray(l2.T), "l2t")
    put(np.ascontiguousarray(U1.T), "u1t")
    put(np.ascontiguousarray(U2.T), "u2t")
    IDPAD = np.zeros((72, CH), np.float32)
    IDPAD[np.arange(72), np.arange(72)] = 1.0
    put(IDPAD, "idpad")

    in_maps = []
    slot_maps = []
    for c in range(NC):
        slots = []
        for a in cores[c]:
            slots.append((a, a))
            for cc in sorted(nbrs2[a]):
                slots.append((a, cc))
        SELT = np.zeros((E, SPAD), np.float32)
        OHSS = np.zeros((72, SPAD), np.float32)
        di = 0
        for s_i, (a, cc) in enumerate(slots):
            SELT[:, s_i] = H2[a] * H2[cc]
            if a == cc:
                OHSS[a, s_i] = 1.0
                di += 1
        pack = base.copy()
        pack[0:72, offs["ohss"]:offs["ohss"] + SPAD] = \
            OHSS.astype(ml_dtypes.bfloat16)
        for q in range(3):
            pack[0:96, offs[f"selt{q}"]:offs[f"selt{q}"] + SPAD] = \
                SELT[96 * q:96 * (q + 1)].astype(ml_dtypes.bfloat16)
        in_maps.append({"PACKB": pack, "KRSD": KRSD})
        slot_maps.append(slots)
    supp_off = np.array([b * (N * N) + d for b, d in supp], np.int64)
    return in_maps, slot_maps, supp_off


_CACHE = {}


def kernel(**inputs):
    from concourse.bass_utils import run_bass_kernel_spmd

    in_maps, slot_maps, supp_off = _prepare(inputs)
    nc = _CACHE.get("nc")
    if nc is None:
        nc = _build_nc()
        _CACHE["nc"] = nc
    res = run_bass_kernel_spmd(nc, in_maps, list(range(NC)))
    nsup = len(supp_off)
    M = np.zeros((N * N, N * N), np.float32)
    for c in range(NC):
        out = res.results[c]["OUT"]
        slots = slot_maps[c]
        bases = np.array([a * (N * N * N) + cc * N for a, cc in slots],
                         np.int64)
        M.flat[bases[:, None] + supp_off[None, :]] = out[:len(slots), :nsup]
    return M


# revision 5
# speedup vs baseline: 2.9501x; 1.1851x over previous
"""Trainium2 Bass kernel for nn_Affinity (gnn_message_passing).

M[(a,b),(c,d)] = sum_{j,i} H2[a,j]H2[c,j] H1[b,i]H1[d,i] W[j,i] + diag(Mp).

Structure exploited:
 - Nonzero blocks (a,c) of M: a==c or (a,c) an edge of graph 2 -> "slots".
   626 slots total, balanced 9 bands/core across 8 cores (<=79 slots/core).
 - Within a block, support is graph-1 adjacency + diagonal: only 622 of 5184
   (b,d) positions can be nonzero, the SAME set for every slot.
 - Per-slot weights w_s[i] = sum_q Me[q,i] SELT[q,s] factor through
   ZS = Xsum^T SELT so the edge-affinity matrix Me is never materialized.
 - Block values OUT[s, f] = sum_i wt[i,s] KRS[i, f] where
   KRS[i,(b,d)] = H1[b,i]H1[d,i] is a host-built 0/1 Khatri-Rao table
   restricted to the support -> one 3-pass matmul chain with partition=slot,
   so results DMA straight out with no transpose/scatter.
 - diag(Mp) folds in as a 4th PSUM accumulation against a [I72|0] table.

All index-derived tables (incidence, SELT, KRS, OHSS, IDPAD) are host-built;
every floating-point op runs on device. Host assembly only places computed
values (and zeros) into the full [5184, 5184] output.
"""
import sys
sys.path.insert(0, '/opt/trn_rl_repo')
import numpy as np

N = 72
E = 288
D = 64
NC = 8
SPAD = 80          # padded slots per core (max observed 79)
NSUP_PAD = 640     # padded support columns (622 actual), 2 chunks of 320
CH = 320           # PSUM free-chunk width


def _split_waits(nc, limit=1):
    """This walrus build rejects instructions with >limit sem waits; move the
    excess onto same-engine NoOps inserted immediately before (same bb order =
    same engine program order, so semantics are preserved)."""
    import concourse.mybir as mybir
    for f in nc.m.functions:
        for bb in f.blocks:
            new_insts = []
            for inst in bb.instructions:
                si = inst.sync_info
                waits = list(si.on_wait) if si and si.on_wait else []
                if len(waits) > limit:
                    extra, keep = waits[:-limit], waits[-limit:]
                    for i in range(0, len(extra), limit):
                        nop = mybir.InstNoOp(
                            name=nc.get_next_instruction_name(),
                            engine=inst.engine, ins=[], outs=[],
                            sync_info=mybir.SyncInfo(
                                on_wait=extra[i:i + limit], on_update=[]),
                        )
                        nc.register_instruction(nop)
                        new_insts.append(nop)
                    si.on_wait = keep
                new_insts.append(inst)
            bb.instructions[:] = new_insts


def _incidence(src, dst):
    H = np.zeros((N, E), np.float32)
    H[src, np.arange(E)] = 1.0
    H[dst, np.arange(E)] = 1.0
    return H


def _neighbors(src, dst):
    nbrs = [set() for _ in range(N)]
    for s, d in zip(src, dst):
        nbrs[int(s)].add(int(d))
        nbrs[int(d)].add(int(s))
    return nbrs


def _plan_assignment(nbrs2):
    """9 bands per core, greedily balancing slot count (1 + deg per band)."""
    deg = [len(x) for x in nbrs2]
    order = sorted(range(N), key=lambda a: -deg[a])
    cores = [[] for _ in range(NC)]
    loads = [0] * NC
    for a in order:
        c = min((c for c in range(NC) if len(cores[c]) < 9),
                key=lambda c: loads[c])
        cores[c].append(a)
        loads[c] += 1 + deg[a]
    assert max(loads) <= SPAD
    return cores


_FIELDS = [("f1", 72, D), ("f2", 72, D), ("h1", 72, E), ("s2", 72, E),
           ("d2m", 72, E), ("l1t", D, D), ("l2t", D, D), ("u1t", D, 72),
           ("u2t", D, 72), ("ohss", 72, SPAD), ("idpad", 72, CH),
           ("selt0", 96, SPAD), ("selt1", 96, SPAD), ("selt2", 96, SPAD)]


def _pk_offs():
    offs, pw = {}, 0
    for nm, r, w in _FIELDS:
        offs[nm] = pw
        pw += w
    return offs, pw


def _build_nc():
    import concourse.bass as bass
    import concourse.mybir as mybir
    import concourse.tile as tile

    F32 = mybir.dt.float32
    BF16 = mybir.dt.bfloat16

    offs, PW = _pk_offs()

    nc = bass.Bass()
    packb_d = nc.declare_dram_parameter("PACKB", [96, PW], BF16, isOutput=False)
    krs_d = nc.declare_dram_parameter("KRSD", [96, 3 * NSUP_PAD], BF16,
                                      isOutput=False)
    out_d = nc.declare_dram_parameter("OUT", [SPAD, NSUP_PAD], F32,
                                      isOutput=True)

    with tile.TileContext(nc) as tc:
        with tc.tile_pool(name="cst", bufs=1) as cst, \
             tc.tile_pool(name="ps", bufs=4, space="PSUM") as ps, \
             tc.tile_pool(name="psb", bufs=4, space="PSUM") as psb:

            pkb = cst.tile([96, PW], BF16)
            nc.sync.dma_start(out=pkb[:], in_=packb_d[:])
            krs = cst.tile([96, 3 * NSUP_PAD], BF16)
            nc.scalar.dma_start(out=krs[:], in_=krs_d[:])

            def fld(nm):
                r, w = next((r, w) for n, r, w in _FIELDS if n == nm)
                return pkb[0:r, offs[nm]:offs[nm] + w]

            f1, f2, h1 = fld("f1"), fld("f2"), fld("h1")
            s2, d2m = fld("s2"), fld("d2m")
            l1t, l2t = fld("l1t"), fld("l2t")
            u1t, u2t = fld("u1t"), fld("u2t")
            ohss, idpad = fld("ohss"), fld("idpad")
            selt = [fld(f"selt{q}") for q in range(3)]

            # relu(lamda^T) tiles (DVE, straight from the input pack)
            r1t = cst.tile([D, D], BF16, tag="r1t")
            nc.vector.tensor_relu(out=r1t[:], in_=l1t)
            r2t = cst.tile([D, D], BF16, tag="r2t")
            nc.vector.tensor_relu(out=r2t[:], in_=l2t)

            # --- PE wave 1: everything that depends only on the input pack.
            # fs = F2^T S2, fdt = F2^T D2M  [D, E]
            fs_p = ps.tile([D, E], F32, tag="mm")
            nc.tensor.matmul(out=fs_p[:], lhsT=f2, rhs=s2, start=True, stop=True)
            fd_p = ps.tile([D, E], F32, tag="mm")
            nc.tensor.matmul(out=fd_p[:], lhsT=f2, rhs=d2m, start=True, stop=True)
            # Xsum chunks: z1tT[q][p, d] = sum_n H1[n, 96q+p] F1[n, d],
            # three 64-wide regions of one PSUM tile -> one copy.
            zp = ps.tile([96, 3 * D], F32, tag="mm")
            for q in range(3):
                nc.tensor.matmul(out=zp[:, D * q:D * (q + 1)],
                                 lhsT=h1[:, 96 * q:96 * (q + 1)], rhs=f1,
                                 start=True, stop=True)
            # Mp = U1 U2^T  [72, 72]
            mp_p = ps.tile([72, 72], F32, tag="mm")
            nc.tensor.matmul(out=mp_p[:], lhsT=u1t, rhs=u2t, start=True, stop=True)

            fsc = cst.tile([D, E], BF16, tag="fsc")
            nc.scalar.copy(out=fsc[:], in_=fs_p[:])
            fdc = cst.tile([D, E], BF16, tag="fdc")
            nc.vector.tensor_copy(out=fdc[:], in_=fd_p[:])
            z1c = cst.tile([96, 3 * D], BF16, tag="z1c")
            nc.vector.tensor_copy(out=z1c[:], in_=zp[:])
            mpc = cst.tile([72, 72], BF16, tag="mpc")
            nc.scalar.copy(out=mpc[:], in_=mp_p[:])

            # --- PE wave 2.
            # vvT chunks: vvT[96q+p, d] = (Ys relu(L1)^T + Yd relu(L2)^T)
            vvt_p = ps.tile([96, 3 * D], F32, tag="mm")
            for q in range(3):
                nc.tensor.matmul(out=vvt_p[:, D * q:D * (q + 1)],
                                 lhsT=fsc[:, 96 * q:96 * (q + 1)], rhs=r1t[:],
                                 start=True, stop=False)
                nc.tensor.matmul(out=vvt_p[:, D * q:D * (q + 1)],
                                 lhsT=fdc[:, 96 * q:96 * (q + 1)], rhs=r2t[:],
                                 start=False, stop=True)
            # ZS[d, s] = sum_q Xsum[q, d] SELT[q, s]  [D, SPAD]
            zs_p = ps.tile([D, SPAD], F32, tag="mm")
            for q in range(3):
                nc.tensor.matmul(out=zs_p[:], lhsT=z1c[:, D * q:D * (q + 1)],
                                 rhs=selt[q], start=(q == 0), stop=(q == 2))
            # mpx[p, s] = Mp[a_s, p] for diag slots (cols of OHSS), else 0
            mpx_p = ps.tile([72, SPAD], F32, tag="mm")
            nc.tensor.matmul(out=mpx_p[:], lhsT=mpc[:], rhs=ohss,
                             start=True, stop=True)

            vvtc = cst.tile([96, 3 * D], BF16, tag="vvtc")
            nc.vector.tensor_copy(out=vvtc[:], in_=vvt_p[:])
            zsc = cst.tile([D, SPAD], BF16, tag="zsc")
            nc.scalar.copy(out=zsc[:], in_=zs_p[:])
            mpxc = cst.tile([72, SPAD], BF16, tag="mpxc")
            nc.scalar.copy(out=mpxc[:], in_=mpx_p[:])

            # --- PE wave 3: VK[d, f] = sum_i vvT[i, d] KRS[i, f]
            vkc = []
            for ch in range(2):
                vk_p = psb.tile([D, CH], F32, tag="bb")
                for q in range(3):
                    nc.tensor.matmul(
                        out=vk_p[:], lhsT=vvtc[:, D * q:D * (q + 1)],
                        rhs=krs[0:96, q * NSUP_PAD + ch * CH:
                                q * NSUP_PAD + (ch + 1) * CH],
                        start=(q == 0), stop=(q == 2))
                vc = cst.tile([D, CH], BF16, tag=f"vkc{ch}")
                if ch == 0:
                    nc.vector.tensor_copy(out=vc[:], in_=vk_p[:])
                else:
                    nc.scalar.copy(out=vc[:], in_=vk_p[:])
                vkc.append(vc)

            # --- PE wave 4: OUT[s, f] = sum_d ZS[d, s] VK[d, f] (+ Mp diag)
            staged = cst.tile([SPAD, NSUP_PAD], F32)
            for ch in range(2):
                bp = psb.tile([SPAD, CH], F32, tag="bb")
                nc.tensor.matmul(out=bp[:], lhsT=zsc[:], rhs=vkc[ch][:],
                                 start=True, stop=(ch == 1))
                if ch == 0:
                    # diag(Mp) add: rhs = [I72 | 0], lhsT = mpx
                    nc.tensor.matmul(out=bp[:], lhsT=mpxc[:], rhs=idpad,
                                     start=False, stop=True)
                if ch == 0:
                    nc.vector.tensor_copy(
                        out=staged[:, ch * CH:(ch + 1) * CH], in_=bp[:])
                else:
                    nc.scalar.copy(
                        out=staged[:, ch * CH:(ch + 1) * CH], in_=bp[:])
                eng = nc.sync if ch == 0 else nc.gpsimd
                eng.dma_start(out=out_d[:, ch * CH:(ch + 1) * CH],
                              in_=staged[:, ch * CH:(ch + 1) * CH])

    _split_waits(nc)
    return nc


def _prepare(inputs):
    import ml_dtypes
    ins = {k: np.asarray(v) for k, v in inputs.items()}
    F1 = ins["F1"].astype(np.float32)
    F2 = ins["F2"].astype(np.float32)
    U1 = ins["U1"].astype(np.float32)
    U2 = ins["U2"].astype(np.float32)
    l1 = ins["lamda1"].astype(np.float32)
    l2 = ins["lamda2"].astype(np.float32)
    src1 = ins["src1"].astype(np.int64)
    dst1 = ins["dst1"].astype(np.int64)
    src2 = ins["src2"].astype(np.int64)
    dst2 = ins["dst2"].astype(np.int64)

    H1 = _incidence(src1, dst1)
    H2 = _incidence(src2, dst2)
    S2 = np.zeros((N, E), np.float32)
    S2[src2, np.arange(E)] = 1.0
    D2M = np.zeros((N, E), np.float32)
    D2M[dst2, np.arange(E)] = 1.0

    nbrs2 = _neighbors(src2, dst2)
    nbrs1 = _neighbors(src1, dst1)
    cores = _plan_assignment(nbrs2)

    # support: diag first (col b = (b,b)), then off-diag adjacency pairs
    supp = [(b, b) for b in range(N)]
    for b in range(N):
        for d in sorted(nbrs1[b]):
            supp.append((b, d))
    nsup = len(supp)
    assert nsup <= NSUP_PAD
    KRS = np.zeros((E, NSUP_PAD), np.float32)
    for f, (b, d) in enumerate(supp):
        KRS[:, f] = H1[b] * H1[d]
    KRSD = np.zeros((96, 3 * NSUP_PAD), ml_dtypes.bfloat16)
    for ic in range(3):
        KRSD[:, ic * NSUP_PAD:(ic + 1) * NSUP_PAD] = \
            KRS[96 * ic:96 * (ic + 1)].astype(ml_dtypes.bfloat16)

    offs, PW = _pk_offs()
    base = np.zeros((96, PW), ml_dtypes.bfloat16)

    def put(arr, nm, r=None):
        rr, w = arr.shape
        base[0:rr, offs[nm]:offs[nm] + w] = arr.astype(ml_dtypes.bfloat16)
    put(F1, "f1")
    put(F2, "f2")
    put(H1, "h1")
    put(S2, "s2")
    put(D2M, "d2m")
    put(np.ascontiguousarray(l1.T), "l1t")
    put(np.ascontiguousarray(l2.T), "l2t")
    put(np.ascontiguousarray(U1.T), "u1t")
    put(np.ascontiguousarray(U2.T), "u2t")
    IDPAD = np.zeros((72, CH), np.float32)
    IDPAD[np.arange(72), np.arange(72)] = 1.0
    put(IDPAD, "idpad")

    in_maps = []
    slot_maps = []
    for c in range(NC):
        slots = []
        for a in cores[c]:
            slots.append((a, a))
            for cc in sorted(nbrs2[a]):
                slots.append((a, cc))
        SELT = np.zeros((E, SPAD), np.float32)
        OHSS = np.zeros((72, SPAD), np.float32)
        di = 0
        for s_i, (a, cc) in enumerate(slots):
            SELT[:, s_i] = H2[a] * H2[cc]
            if a == cc:
                OHSS[a, s_i] = 1.0
                di += 1
        pack = base.copy()
        pack[0:72, offs["ohss"]:offs["ohss"] + SPAD] = \
            OHSS.astype(ml_dtypes.bfloat16)
        for q in range(3):
            pack[0:96, offs[f"selt{q}"]:offs[f"selt{q}"] + SPAD] = \
                SELT[96 * q:96 * (q + 1)].astype(ml_dtypes.bfloat16)
        in_maps.append({"PACKB": pack, "KRSD": KRSD})
        slot_maps.append(slots)
    supp_off = np.array([b * (N * N) + d for b, d in supp], np.int64)
    return in_maps, slot_maps, supp_off


_CACHE = {}


def kernel(**inputs):
    from concourse.bass_utils import run_bass_kernel_spmd

    in_maps, slot_maps, supp_off = _prepare(inputs)
    nc = _CACHE.get("nc")
    if nc is None:
        nc = _build_nc()
        _CACHE["nc"] = nc
    res = run_bass_kernel_spmd(nc, in_maps, list(range(NC)))
    nsup = len(supp_off)
    M = np.zeros((N * N, N * N), np.float32)
    for c in range(NC):
        out = res.results[c]["OUT"]
        slots = slot_maps[c]
        bases = np.array([a * (N * N * N) + cc * N for a, cc in slots],
                         np.int64)
        M.flat[bases[:, None] + supp_off[None, :]] = out[:len(slots), :nsup]
    return M
